# revision 38
# baseline (speedup 1.0000x reference)
"""Trainium2 Bass kernel for a GRU "communication head".

Model (per reference):
    h0 = state @ Wp.T + bp                    (B, H)
    xs = embed[target]                        (B, T, E)
    for t: h = GRUCell(xs[:, t], h); logits_t = h @ Wo.T + bo
    out = stack logits                        (B, T, V)

Shapes: B=32, T=64, E=64, H=256, V=32003, INPUT_DIM=512.

Strategy (8 NeuronCores):
  - Vocab(column)-parallel: V padded to 32768 = 8 * 4096. Each core holds a
    4096-wide slice of Wo/bo and produces logits for ALL (t, b) positions of
    its slice. The tiny GRU recurrence is computed redundantly on every core.
  - Transposed on-chip layout: hidden states live as H_allT [128, 2, T*B]
    (f32r) so the output projection is a dense fp32r matmul (full PE rate)
    streamed to HBM as it becomes available.
  - Output is vocab-major ([v, t*B+b] per core) so bo is a per-partition
    bias fused for free into the PSUM->SBUF copy (DVE tensor_scalar_add /
    ACT Identity+bias, alternating to balance the two engines).
  - The recurrence runs as TWO independent 16-wide batch streams,
    software-pipelined at half-step "tick" granularity so neither stream
    head-of-line blocks the other on the in-order engine queues; the
    h-side (W_hh @ h) matmuls consume H_allT directly in f32r.
  - Gate pre-activations accumulate in one PSUM bank per (step, stream):
    slots [r, z, h_n, i_n] x 2 H-chunks x 16. A K=1 bf16 "zero opener"
    matmul clears the bank and orders the accumulation; x-side fp32
    contributions (incl. all biases via an appended ones-row on the
    embedded inputs) and the h-side matmuls then accumulate.
  - Output-projection tiles are emitted interleaved (~1.5 per tick) so the
    in-order PE queue never stalls the recurrence; first/last waves are
    256 columns so the output DMA stream starts early and drains early.
"""

import numpy as np
import ml_dtypes

B = 32
SB = 16  # per-stream batch
T = 64
E = 64
H = 256
V = 32003
INPUT_DIM = 512
NCORES = 8
VPAD = 32768
VLOC = VPAD // NCORES  # 4096
TB = T * B  # 2048
KST = 640  # padded (INPUT_DIM + bias row) -> 5 chunks of 128
NVCH = VLOC // 128  # 32 vocab chunks (output partition dim)
DMA_GROUP = 4  # vocab chunks per output DMA
# (start_col, ncols) waves; ends when the step producing its last column done
WAVES = [(0, 256), (256, 256), (512, 512), (1024, 512), (1536, 256), (1792, 256)]

_CACHE = {}


def _build_nc():
    import concourse.mybir as mybir
    import concourse.tile as tile
    from concourse import bacc

    f32 = mybir.dt.float32
    f32r = mybir.dt.float32r
    bf16 = mybir.dt.bfloat16
    AF = mybir.ActivationFunctionType

    nc = bacc.Bacc(
        "TRN2",
        debug=False,
        enable_asserts=False,
        target_bir_lowering=False,
        num_devices=NCORES,
    )

    d_xsT = nc.dram_tensor("xsT", (128, TB), f32, kind="ExternalInput")
    d_wihT = nc.dram_tensor("wihT", (128, 3 * H), f32, kind="ExternalInput")
    d_whhT = nc.dram_tensor("whhT", (H, 3 * H), f32r, kind="ExternalInput")
    d_bhhn = nc.dram_tensor("bhhn", (1, H), f32, kind="ExternalInput")
    d_stT = nc.dram_tensor("stT", (KST, B), f32, kind="ExternalInput")
    d_wpT = nc.dram_tensor("wpT", (KST, H), f32, kind="ExternalInput")
    d_woT = nc.dram_tensor("woT", (H, VLOC), f32r, kind="ExternalInput")
    # bo regrouped host-side to [128, NVCH]: column m holds bo[m*128:(m+1)*128]
    d_bo = nc.dram_tensor("bo", (128, NVCH), f32, kind="ExternalInput")
    # [1, 256] bf16: cols 0:128 zeros, 128:256 ones (for the PSUM openers)
    d_cb = nc.dram_tensor("cb", (1, 256), bf16, kind="ExternalInput")
    # vocab-major output: row v (local), col t*B+b
    d_out = nc.dram_tensor("out", (VLOC, TB), f32, kind="ExternalOutput")

    with tile.TileContext(nc) as tc:
        with (
            tc.tile_pool(name="weights", bufs=1) as wpool,
            tc.tile_pool(name="state", bufs=1) as spool,
            tc.tile_pool(name="gates_ps", bufs=2, space="PSUM") as gps,
            tc.tile_pool(name="logit_ps", bufs=4, space="PSUM") as lps,
            tc.tile_pool(name="tmp", bufs=4) as tmp,
            tc.tile_pool(name="ostage", bufs=3) as ost,
        ):
            # ---- persistent SBUF loads ----
            # order matters: the opener constants (cb) and the recurrence
            # weights must not queue behind the 4MB woT load, or the first
            # recurrence steps stall ~28us waiting for them
            cb = wpool.tile([1, 256], bf16, tag="cb")
            nc.sync.dma_start(out=cb, in_=d_cb.ap())
            stT = wpool.tile([128, 5, B], f32, tag="stT")
            nc.sync.dma_start(
                out=stT, in_=d_stT.ap().rearrange("(kc p) b -> p kc b", p=128)
            )
            wpT = wpool.tile([128, 5, H], f32, tag="wpT")
            nc.sync.dma_start(
                out=wpT, in_=d_wpT.ap().rearrange("(kc p) m -> p kc m", p=128)
            )
            bhhn = wpool.tile([1, H], f32, tag="bhhn")
            nc.sync.dma_start(out=bhhn, in_=d_bhhn.ap())
            wihT = wpool.tile([128, 3 * H], f32, tag="wihT")
            nc.sync.dma_start(out=wihT, in_=d_wihT.ap())
            whhT = wpool.tile([128, 2, 3 * H], f32r, tag="whhT")
            nc.sync.dma_start(
                out=whhT, in_=d_whhT.ap().rearrange("(kc p) m -> p kc m", p=128)
            )
            xsT = wpool.tile([128, TB], f32, tag="xsT")
            for c4 in range(4):
                cs4 = slice(c4 * (TB // 4), (c4 + 1) * (TB // 4))
                nc.sync.dma_start(out=xsT[:, cs4], in_=d_xsT.ap()[:, cs4])
            bo = wpool.tile([128, NVCH], f32, tag="bo")
            nc.sync.dma_start(out=bo, in_=d_bo.ap())
            woT = wpool.tile([128, 2, VLOC], f32r, tag="woT")
            woT_src = d_woT.ap().rearrange("(kc p) v -> p kc v", p=128)
            for c4 in range(4):
                vs4 = slice(c4 * (VLOC // 4), (c4 + 1) * (VLOC // 4))
                nc.sync.dma_start(out=woT[:, :, vs4], in_=woT_src[:, :, vs4])
            zcol_b = cb[0:1, 0:128]
            ones_b = cb[0:1, 128:256]

            ones = wpool.tile([1, 256], f32, tag="ones")
            nc.vector.memset(ones, 1.0)

            H_allT = spool.tile([128, 2, TB], f32r, tag="H_allT")
            h0 = spool.tile([128, 2, B], f32r, tag="h0")

            # vocab-major view of the output for grouped DMA
            out_g = d_out.ap().rearrange("(g p) t -> p g t", p=128)

            # ---- h0 = state @ Wp.T + bp (bias folded into padded row 512) ----
            ps0 = gps.tile([128, 2, B], f32, tag="g0")
            nc.tensor.matmul(
                ps0, zcol_b, ones_b[0:1, 0:64], start=True, stop=False,
                skip_group_check=True,
            )
            for ko in range(2):
                for ki in range(5):
                    nc.tensor.matmul(
                        ps0[:, ko, :],
                        wpT[:, ki, ko * 128 : (ko + 1) * 128],
                        stT[:, ki, :],
                        start=False,
                        stop=(ko == 1 and ki == 4),
                        skip_group_check=True,
                    )
            nc.any.tensor_copy(h0, ps0)

            # ---- output-projection tile machinery ----
            # matmuls are emitted at tick k; the PSUM->SBUF copy (+bias) and
            # the grouped DMA are deferred to tick k+1 so they enter the
            # DVE/ACT queues with their dependencies already satisfied
            # (otherwise they convoy the next tick's recurrence ops).
            state_d = {"flip": 0, "ob": None}
            pending = []  # (start_col, ncols, m) logit tiles ready to emit
            deferred = []  # (ps, m, tbs, ncols) awaiting copy emission

            def emit_logit_mm():
                if not pending:
                    return
                c0, ncols, m = pending.pop(0)
                tbs = slice(c0, c0 + ncols)
                ps = lps.tile([128, ncols], f32, tag="lg", name="lg")
                nc.tensor.matmul(
                    ps,
                    woT[:, 0, m * 128 : (m + 1) * 128],
                    H_allT[:, 0, tbs],
                    start=True,
                    stop=False,
                )
                nc.tensor.matmul(
                    ps,
                    woT[:, 1, m * 128 : (m + 1) * 128],
                    H_allT[:, 1, tbs],
                    start=False,
                    stop=True,
                )
                deferred.append((ps, m, tbs, ncols))

            def emit_logit_copies():
                while deferred:
                    ps, m, tbs, ncols = deferred.pop(0)
                    g, j = divmod(m, DMA_GROUP)
                    if j == 0:
                        state_d["ob"] = ost.tile(
                            [128, DMA_GROUP, ncols], f32, tag="ob", name="ob"
                        )
                    ob = state_d["ob"]
                    bias_ap = bo[:, m : m + 1]
                    if state_d["flip"] % 3 == 0:
                        nc.vector.tensor_scalar_add(ob[:, j, :], ps, bias_ap)
                    else:
                        nc.scalar.activation(
                            out=ob[:, j, :], in_=ps, func=AF.Identity, bias=bias_ap
                        )
                    state_d["flip"] += 1
                    if j == DMA_GROUP - 1:
                        nc.sync.dma_start(
                            out=out_g[
                                :, g * DMA_GROUP : (g + 1) * DMA_GROUP, tbs
                            ],
                            in_=ob,
                        )

            wave_by_end_step = {
                (c0 + ncols) // B - 1: (c0, ncols) for c0, ncols in WAVES
            }

            # ---- recurrence: 2 streams, software-pipelined over "ticks" ----
            # tick k handles stream k%2, step k//2. Per tick we emit, in
            # order: the POST phase (tanh..h-write) of tick k-2, the PRE
            # phase (sigmoid..b2) of tick k-1, and the gate matmuls of tick
            # k. This staggers the two streams' chains by one tick so the
            # in-order engine queues never head-of-line block, and copies
            # slot into the ACT/DVE bubbles.
            NTICK = 2 * T
            pre_st = {}  # tick -> (P, rz, b2, hprev_f, cs)

            def col_slice(k):
                sg, t = k % 2, k // 2
                return slice(t * B + sg * SB, t * B + (sg + 1) * SB)

            def emit_gates(k):
                sg, t = k % 2, k // 2
                cs = col_slice(k)
                if t == 0:
                    hprev_r = h0[:, :, sg * SB : (sg + 1) * SB]
                else:
                    hprev_r = H_allT[
                        :, :, (t - 1) * B + sg * SB : (t - 1) * B + (sg + 1) * SB
                    ]
                hprev_f = hprev_r.bitcast(f32)
                P = gps.tile([128, 4, 2, SB], f32, tag=f"g{sg}", name=f"P{sg}")
                nc.tensor.matmul(
                    P, zcol_b, ones_b, start=True, stop=False,
                    skip_group_check=True,
                )
                for s, slot in ((0, 0), (1, 1), (2, 3)):
                    for ko in range(2):
                        blk = s * 2 + ko
                        nc.tensor.matmul(
                            P[:, slot, ko, :],
                            wihT[:, blk * 128 : (blk + 1) * 128],
                            xsT[:, cs],
                            start=False,
                            stop=False,
                            skip_group_check=True,
                        )
                for ko in range(2):
                    nc.tensor.matmul(
                        P[:, 2, ko, :],
                        bhhn[0:1, ko * 128 : (ko + 1) * 128],
                        ones[0:1, 0:SB],
                        start=False,
                        stop=False,
                        skip_group_check=True,
                    )
                for s in range(3):
                    slot = s if s < 2 else 2
                    for ko in range(2):
                        blk = s * 2 + ko
                        for ki in range(2):
                            nc.tensor.matmul(
                                P[:, slot, ko, :],
                                whhT[:, ki, blk * 128 : (blk + 1) * 128],
                                hprev_r[:, ki, :],
                                start=False,
                                stop=(s == 2 and ko == 1 and ki == 1),
                                skip_group_check=True,
                            )
                pre_st[k] = (P, hprev_f, cs)

            def emit_pre(k):
                sg = k % 2
                P, hprev_f, cs = pre_st[k]
                rz = tmp.tile([128, 2, 2, SB], f32, tag=f"rz{sg}", name=f"rz{sg}")
                nc.scalar.activation(out=rz, in_=P[:, 0:2, :, :], func=AF.Sigmoid)
                a = tmp.tile([128, 2, SB], f32, tag=f"a{sg}", name=f"a{sg}")
                nc.vector.tensor_mul(a, rz[:, 0, :, :], P[:, 2, :, :])
                b2 = tmp.tile([128, 2, SB], f32, tag=f"b2{sg}", name=f"b2{sg}")
                nc.vector.tensor_add(b2, a, P[:, 3, :, :])
                q = tmp.tile([128, 2, SB], f32, tag=f"q{sg}", name=f"q{sg}")
                nc.gpsimd.tensor_mul(q, rz[:, 1, :, :], hprev_f)
                om = tmp.tile([128, 2, SB], f32, tag=f"om{sg}", name=f"om{sg}")
                nc.gpsimd.tensor_scalar(om, rz[:, 1, :, :], -1.0, 1.0,
                                        mybir.AluOpType.mult, mybir.AluOpType.add)
                pre_st[k] = (P, b2, q, om, cs)

            def emit_post(k):
                sg = k % 2
                P, b2, q, om, cs = pre_st.pop(k)
                nsb = tmp.tile([128, 2, SB], f32, tag=f"nsb{sg}", name=f"nsb{sg}")
                nc.scalar.activation(out=nsb, in_=b2, func=AF.Tanh)
                v = tmp.tile([128, 2, SB], f32, tag=f"v{sg}", name=f"v{sg}")
                nc.vector.tensor_mul(v, nsb, om)
                # single f32r master copy: consumed by the next step's
                # h-side matmuls AND the output projection
                nc.vector.tensor_add(H_allT[:, :, cs], v, q)

            for k in range(NTICK + 2):
                if k >= 2:
                    emit_post(k - 2)
                    sg, t = (k - 2) % 2, (k - 2) // 2
                    if sg == 1 and t in wave_by_end_step:
                        c0, ncols = wave_by_end_step[t]
                        pending.extend((c0, ncols, m) for m in range(NVCH))
                emit_logit_copies()
                if 1 <= k <= NTICK:
                    emit_pre(k - 1)
                if k < NTICK:
                    emit_gates(k)
                # ~1.5 logit tiles per tick keeps PE dense without stalling
                # the in-order recurrence matmuls behind a whole wave
                emit_logit_mm()
                if k % 2 == 0:
                    emit_logit_mm()

            while pending:
                emit_logit_mm()
                emit_logit_copies()
            emit_logit_copies()

    nc.compile()
    return nc


def _get_nc():
    if "nc" not in _CACHE:
        _CACHE["nc"] = _build_nc()
    return _CACHE["nc"]


def _prep_in_maps(state, target, embed, Wp, bp, W_ih, W_hh, b_ih, b_hh, Wo, bo):
    f = np.float32
    state = np.asarray(state, dtype=f)
    target = np.asarray(target)
    embed = np.asarray(embed, dtype=f)
    Wp = np.asarray(Wp, dtype=f)
    bp = np.asarray(bp, dtype=f)
    W_ih = np.asarray(W_ih, dtype=f)
    W_hh = np.asarray(W_hh, dtype=f)
    b_ih = np.asarray(b_ih, dtype=f)
    b_hh = np.asarray(b_hh, dtype=f)
    Wo = np.asarray(Wo, dtype=f)
    bo = np.asarray(bo, dtype=f)

    # host-side gather + transpose to (E, T*B), col = t*B + b
    xs = embed[target.astype(np.int64)]  # (B, T, E)
    xsT = np.ascontiguousarray(xs.transpose(1, 0, 2).reshape(TB, E).T)  # (E, TB)
    xsT_pad = np.zeros((128, TB), f)
    xsT_pad[:E] = xsT
    xsT_pad[E] = 1.0  # bias row

    bias_gi = np.concatenate([b_ih[: 2 * H] + b_hh[: 2 * H], b_ih[2 * H :]])
    wihT_pad = np.zeros((128, 3 * H), f)
    wihT_pad[:E] = W_ih.T
    wihT_pad[E] = bias_gi

    whhT = np.ascontiguousarray(W_hh.T)  # (H, 3H)
    bhhn = np.ascontiguousarray(b_hh[2 * H :][None, :])  # (1, H)

    stT_pad = np.zeros((KST, B), f)
    stT_pad[:INPUT_DIM] = state.T
    stT_pad[INPUT_DIM] = 1.0
    wpT_pad = np.zeros((KST, H), f)
    wpT_pad[:INPUT_DIM] = Wp.T
    wpT_pad[INPUT_DIM] = bp

    woT_full = np.zeros((H, VPAD), f)
    woT_full[:, :V] = Wo.T
    bo_full = np.zeros((VPAD,), f)
    bo_full[:V] = bo

    cb = np.zeros((1, 256), ml_dtypes.bfloat16)
    cb[0, 128:] = 1.0

    in_maps = []
    for c in range(NCORES):
        vs = slice(c * VLOC, (c + 1) * VLOC)
        in_maps.append(
            {
                "xsT": xsT_pad,
                "wihT": wihT_pad,
                "whhT": whhT,
                "bhhn": bhhn,
                "stT": stT_pad,
                "wpT": wpT_pad,
                "woT": np.ascontiguousarray(woT_full[:, vs]),
                "bo": np.ascontiguousarray(bo_full[vs].reshape(NVCH, 128).T),
                "cb": cb,
            }
        )
    return in_maps


def _assemble(results):
    full = np.concatenate([r["out"] for r in results], axis=0)  # (VPAD, TB)
    # out[b, t, v] = full[v, t*B + b]
    out = full[:V].reshape(V, T, B).transpose(2, 1, 0)
    return np.ascontiguousarray(out)


def _run(in_maps, **kwargs):
    from concourse.bass_utils import run_bass_kernel_spmd

    nc = _get_nc()
    return run_bass_kernel_spmd(nc, in_maps, core_ids=list(range(NCORES)), **kwargs)


def kernel(**inputs):
    in_maps = _prep_in_maps(**inputs)
    res = _run(in_maps)
    return _assemble(res.results)


# revision 40
# speedup vs baseline: 1.0067x; 1.0067x over previous
"""Trainium2 Bass kernel for a GRU "communication head".

Model (per reference):
    h0 = state @ Wp.T + bp                    (B, H)
    xs = embed[target]                        (B, T, E)
    for t: h = GRUCell(xs[:, t], h); logits_t = h @ Wo.T + bo
    out = stack logits                        (B, T, V)

Shapes: B=32, T=64, E=64, H=256, V=32003, INPUT_DIM=512.

Strategy (8 NeuronCores):
  - Vocab(column)-parallel: V padded to 32768 = 8 * 4096. Each core holds a
    4096-wide slice of Wo/bo and produces logits for ALL (t, b) positions of
    its slice. The tiny GRU recurrence is computed redundantly on every core.
  - Transposed on-chip layout: hidden states live as H_allT [128, 2, T*B]
    (f32r) so the output projection is a dense fp32r matmul (full PE rate)
    streamed to HBM as it becomes available.
  - Output is vocab-major ([v, t*B+b] per core) so bo is a per-partition
    bias fused for free into the PSUM->SBUF copy (DVE tensor_scalar_add /
    ACT Identity+bias, alternating to balance the two engines).
  - The recurrence runs as TWO independent 16-wide batch streams,
    software-pipelined at half-step "tick" granularity so neither stream
    head-of-line blocks the other on the in-order engine queues; the
    h-side (W_hh @ h) matmuls consume H_allT directly in f32r.
  - Gate pre-activations accumulate in one PSUM bank per (step, stream):
    slots [r, z, h_n, i_n] x 2 H-chunks x 16. A K=1 bf16 "zero opener"
    matmul clears the bank and orders the accumulation; x-side fp32
    contributions (incl. all biases via an appended ones-row on the
    embedded inputs) and the h-side matmuls then accumulate.
  - Output-projection tiles are emitted interleaved (~1.5 per tick) so the
    in-order PE queue never stalls the recurrence; first/last waves are
    256 columns so the output DMA stream starts early and drains early.
"""

import numpy as np
import ml_dtypes

B = 32
SB = 16  # per-stream batch
T = 64
E = 64
H = 256
V = 32003
INPUT_DIM = 512
NCORES = 8
VPAD = 32768
VLOC = VPAD // NCORES  # 4096
TB = T * B  # 2048
KST = 640  # padded (INPUT_DIM + bias row) -> 5 chunks of 128
NVCH = VLOC // 128  # 32 vocab chunks (output partition dim)
DMA_GROUP = 4  # vocab chunks per output DMA
# (start_col, ncols) waves; ends when the step producing its last column done
WAVES = [(0, 256), (256, 256), (512, 512), (1024, 512), (1536, 256), (1792, 256)]

_CACHE = {}


def _build_nc():
    import concourse.mybir as mybir
    import concourse.tile as tile
    from concourse import bacc

    f32 = mybir.dt.float32
    f32r = mybir.dt.float32r
    bf16 = mybir.dt.bfloat16
    AF = mybir.ActivationFunctionType

    nc = bacc.Bacc(
        "TRN2",
        debug=False,
        enable_asserts=False,
        target_bir_lowering=False,
        num_devices=NCORES,
    )

    d_xsT = nc.dram_tensor("xsT", (128, TB), f32, kind="ExternalInput")
    d_wihT = nc.dram_tensor("wihT", (128, 3 * H), f32, kind="ExternalInput")
    d_whhT = nc.dram_tensor("whhT", (H, 3 * H), f32r, kind="ExternalInput")
    d_bhhn = nc.dram_tensor("bhhn", (1, H), f32, kind="ExternalInput")
    d_stT = nc.dram_tensor("stT", (KST, B), f32, kind="ExternalInput")
    d_wpT = nc.dram_tensor("wpT", (KST, H), f32, kind="ExternalInput")
    d_woT = nc.dram_tensor("woT", (H, VLOC), f32r, kind="ExternalInput")
    # bo regrouped host-side to [128, NVCH]: column m holds bo[m*128:(m+1)*128]
    d_bo = nc.dram_tensor("bo", (128, NVCH), f32, kind="ExternalInput")
    # [1, 256] bf16: cols 0:128 zeros, 128:256 ones (for the PSUM openers)
    d_cb = nc.dram_tensor("cb", (1, 256), bf16, kind="ExternalInput")
    # vocab-major output: row v (local), col t*B+b
    d_out = nc.dram_tensor("out", (VLOC, TB), f32, kind="ExternalOutput")

    with tile.TileContext(nc) as tc:
        with (
            tc.tile_pool(name="weights", bufs=1) as wpool,
            tc.tile_pool(name="state", bufs=1) as spool,
            tc.tile_pool(name="gates_ps", bufs=2, space="PSUM") as gps,
            tc.tile_pool(name="logit_ps", bufs=4, space="PSUM") as lps,
            tc.tile_pool(name="tmp", bufs=4) as tmp,
            tc.tile_pool(name="ostage", bufs=4) as ost,
        ):
            # ---- persistent SBUF loads ----
            # order matters: the opener constants (cb) and the recurrence
            # weights must not queue behind the 4MB woT load, or the first
            # recurrence steps stall ~28us waiting for them
            cb = wpool.tile([1, 256], bf16, tag="cb")
            nc.sync.dma_start(out=cb, in_=d_cb.ap())
            stT = wpool.tile([128, 5, B], f32, tag="stT")
            nc.sync.dma_start(
                out=stT, in_=d_stT.ap().rearrange("(kc p) b -> p kc b", p=128)
            )
            wpT = wpool.tile([128, 5, H], f32, tag="wpT")
            nc.sync.dma_start(
                out=wpT, in_=d_wpT.ap().rearrange("(kc p) m -> p kc m", p=128)
            )
            bhhn = wpool.tile([1, H], f32, tag="bhhn")
            nc.sync.dma_start(out=bhhn, in_=d_bhhn.ap())
            wihT = wpool.tile([128, 3 * H], f32, tag="wihT")
            nc.sync.dma_start(out=wihT, in_=d_wihT.ap())
            whhT = wpool.tile([128, 2, 3 * H], f32r, tag="whhT")
            nc.sync.dma_start(
                out=whhT, in_=d_whhT.ap().rearrange("(kc p) m -> p kc m", p=128)
            )
            xsT = wpool.tile([128, TB], f32, tag="xsT")
            for c4 in range(4):
                cs4 = slice(c4 * (TB // 4), (c4 + 1) * (TB // 4))
                nc.sync.dma_start(out=xsT[:, cs4], in_=d_xsT.ap()[:, cs4])
            bo = wpool.tile([128, NVCH], f32, tag="bo")
            nc.sync.dma_start(out=bo, in_=d_bo.ap())
            woT = wpool.tile([128, 2, VLOC], f32r, tag="woT")
            woT_src = d_woT.ap().rearrange("(kc p) v -> p kc v", p=128)
            for c4 in range(4):
                vs4 = slice(c4 * (VLOC // 4), (c4 + 1) * (VLOC // 4))
                nc.sync.dma_start(out=woT[:, :, vs4], in_=woT_src[:, :, vs4])
            zcol_b = cb[0:1, 0:128]
            ones_b = cb[0:1, 128:256]

            ones = wpool.tile([1, 256], f32, tag="ones")
            nc.vector.memset(ones, 1.0)

            H_allT = spool.tile([128, 2, TB], f32r, tag="H_allT")
            h0 = spool.tile([128, 2, B], f32r, tag="h0")

            # vocab-major view of the output for grouped DMA
            out_g = d_out.ap().rearrange("(g p) t -> p g t", p=128)

            # ---- h0 = state @ Wp.T + bp (bias folded into padded row 512) ----
            ps0 = gps.tile([128, 2, B], f32, tag="g0")
            nc.tensor.matmul(
                ps0, zcol_b, ones_b[0:1, 0:64], start=True, stop=False,
                skip_group_check=True,
            )
            for ko in range(2):
                for ki in range(5):
                    nc.tensor.matmul(
                        ps0[:, ko, :],
                        wpT[:, ki, ko * 128 : (ko + 1) * 128],
                        stT[:, ki, :],
                        start=False,
                        stop=(ko == 1 and ki == 4),
                        skip_group_check=True,
                    )
            nc.any.tensor_copy(h0, ps0)

            # ---- output-projection tile machinery ----
            # matmuls are emitted at tick k; the PSUM->SBUF copy (+bias) and
            # the grouped DMA are deferred to tick k+1 so they enter the
            # DVE/ACT queues with their dependencies already satisfied
            # (otherwise they convoy the next tick's recurrence ops).
            state_d = {"flip": 0, "ob": None}
            pending = []  # (start_col, ncols, m) logit tiles ready to emit
            deferred = []  # (ps, m, tbs, ncols) awaiting copy emission

            def emit_logit_mm():
                if not pending:
                    return
                c0, ncols, m = pending.pop(0)
                tbs = slice(c0, c0 + ncols)
                ps = lps.tile([128, ncols], f32, tag="lg", name="lg")
                nc.tensor.matmul(
                    ps,
                    woT[:, 0, m * 128 : (m + 1) * 128],
                    H_allT[:, 0, tbs],
                    start=True,
                    stop=False,
                )
                nc.tensor.matmul(
                    ps,
                    woT[:, 1, m * 128 : (m + 1) * 128],
                    H_allT[:, 1, tbs],
                    start=False,
                    stop=True,
                )
                deferred.append((ps, m, tbs, ncols))

            def emit_logit_copies():
                while deferred:
                    ps, m, tbs, ncols = deferred.pop(0)
                    g, j = divmod(m, DMA_GROUP)
                    if j == 0:
                        state_d["ob"] = ost.tile(
                            [128, DMA_GROUP, ncols], f32, tag="ob", name="ob"
                        )
                    ob = state_d["ob"]
                    bias_ap = bo[:, m : m + 1]
                    if state_d["flip"] % 3 == 0:
                        nc.vector.tensor_scalar_add(ob[:, j, :], ps, bias_ap)
                    else:
                        nc.scalar.activation(
                            out=ob[:, j, :], in_=ps, func=AF.Identity, bias=bias_ap
                        )
                    state_d["flip"] += 1
                    if j == DMA_GROUP - 1:
                        nc.sync.dma_start(
                            out=out_g[
                                :, g * DMA_GROUP : (g + 1) * DMA_GROUP, tbs
                            ],
                            in_=ob,
                        )

            wave_by_end_step = {
                (c0 + ncols) // B - 1: (c0, ncols) for c0, ncols in WAVES
            }

            # ---- recurrence: 2 streams, software-pipelined over "ticks" ----
            # tick k handles stream k%2, step k//2. Per tick we emit, in
            # order: the POST phase (tanh..h-write) of tick k-2, the PRE
            # phase (sigmoid..b2) of tick k-1, and the gate matmuls of tick
            # k. This staggers the two streams' chains by one tick so the
            # in-order engine queues never head-of-line block, and copies
            # slot into the ACT/DVE bubbles.
            NTICK = 2 * T
            pre_st = {}  # tick -> (P, rz, b2, hprev_f, cs)

            def col_slice(k):
                sg, t = k % 2, k // 2
                return slice(t * B + sg * SB, t * B + (sg + 1) * SB)

            def emit_gates(k):
                sg, t = k % 2, k // 2
                cs = col_slice(k)
                if t == 0:
                    hprev_r = h0[:, :, sg * SB : (sg + 1) * SB]
                else:
                    hprev_r = H_allT[
                        :, :, (t - 1) * B + sg * SB : (t - 1) * B + (sg + 1) * SB
                    ]
                hprev_f = hprev_r.bitcast(f32)
                P = gps.tile([128, 4, 2, SB], f32, tag=f"g{sg}", name=f"P{sg}")
                nc.tensor.matmul(
                    P, zcol_b, ones_b, start=True, stop=False,
                    skip_group_check=True,
                )
                for s, slot in ((0, 0), (1, 1), (2, 3)):
                    for ko in range(2):
                        blk = s * 2 + ko
                        nc.tensor.matmul(
                            P[:, slot, ko, :],
                            wihT[:, blk * 128 : (blk + 1) * 128],
                            xsT[:, cs],
                            start=False,
                            stop=False,
                            skip_group_check=True,
                        )
                for ko in range(2):
                    nc.tensor.matmul(
                        P[:, 2, ko, :],
                        bhhn[0:1, ko * 128 : (ko + 1) * 128],
                        ones[0:1, 0:SB],
                        start=False,
                        stop=False,
                        skip_group_check=True,
                    )
                for s in range(3):
                    slot = s if s < 2 else 2
                    for ko in range(2):
                        blk = s * 2 + ko
                        for ki in range(2):
                            nc.tensor.matmul(
                                P[:, slot, ko, :],
                                whhT[:, ki, blk * 128 : (blk + 1) * 128],
                                hprev_r[:, ki, :],
                                start=False,
                                stop=(s == 2 and ko == 1 and ki == 1),
                                skip_group_check=True,
                            )
                pre_st[k] = (P, hprev_f, cs)

            def emit_pre(k):
                sg = k % 2
                P, hprev_f, cs = pre_st[k]
                rz = tmp.tile([128, 2, 2, SB], f32, tag=f"rz{sg}", name=f"rz{sg}")
                nc.scalar.activation(out=rz, in_=P[:, 0:2, :, :], func=AF.Sigmoid)
                a = tmp.tile([128, 2, SB], f32, tag=f"a{sg}", name=f"a{sg}")
                nc.vector.tensor_mul(a, rz[:, 0, :, :], P[:, 2, :, :])
                b2 = tmp.tile([128, 2, SB], f32, tag=f"b2{sg}", name=f"b2{sg}")
                nc.vector.tensor_add(b2, a, P[:, 3, :, :])
                q = tmp.tile([128, 2, SB], f32, tag=f"q{sg}", name=f"q{sg}")
                nc.gpsimd.tensor_mul(q, rz[:, 1, :, :], hprev_f)
                om = tmp.tile([128, 2, SB], f32, tag=f"om{sg}", name=f"om{sg}")
                nc.gpsimd.tensor_scalar(om, rz[:, 1, :, :], -1.0, 1.0,
                                        mybir.AluOpType.mult, mybir.AluOpType.add)
                pre_st[k] = (P, b2, q, om, cs)

            def emit_post(k):
                sg = k % 2
                P, b2, q, om, cs = pre_st.pop(k)
                nsb = tmp.tile([128, 2, SB], f32, tag=f"nsb{sg}", name=f"nsb{sg}")
                nc.scalar.activation(out=nsb, in_=b2, func=AF.Tanh)
                v = tmp.tile([128, 2, SB], f32, tag=f"v{sg}", name=f"v{sg}")
                nc.vector.tensor_mul(v, nsb, om)
                # single f32r master copy: consumed by the next step's
                # h-side matmuls AND the output projection
                nc.vector.tensor_add(H_allT[:, :, cs], v, q)

            for k in range(NTICK + 2):
                if k >= 2:
                    emit_post(k - 2)
                    sg, t = (k - 2) % 2, (k - 2) // 2
                    if sg == 1 and t in wave_by_end_step:
                        c0, ncols = wave_by_end_step[t]
                        pending.extend((c0, ncols, m) for m in range(NVCH))
                emit_logit_copies()
                if 1 <= k <= NTICK:
                    emit_pre(k - 1)
                if k < NTICK:
                    emit_gates(k)
                # ~1.5 logit tiles per tick keeps PE dense without stalling
                # the in-order recurrence matmuls behind a whole wave
                emit_logit_mm()
                if k % 2 == 0:
                    emit_logit_mm()

            while pending:
                emit_logit_mm()
                emit_logit_copies()
            emit_logit_copies()

    nc.compile()
    return nc


def _get_nc():
    if "nc" not in _CACHE:
        _CACHE["nc"] = _build_nc()
    return _CACHE["nc"]


def _prep_in_maps(state, target, embed, Wp, bp, W_ih, W_hh, b_ih, b_hh, Wo, bo):
    f = np.float32
    state = np.asarray(state, dtype=f)
    target = np.asarray(target)
    embed = np.asarray(embed, dtype=f)
    Wp = np.asarray(Wp, dtype=f)
    bp = np.asarray(bp, dtype=f)
    W_ih = np.asarray(W_ih, dtype=f)
    W_hh = np.asarray(W_hh, dtype=f)
    b_ih = np.asarray(b_ih, dtype=f)
    b_hh = np.asarray(b_hh, dtype=f)
    Wo = np.asarray(Wo, dtype=f)
    bo = np.asarray(bo, dtype=f)

    # host-side gather + transpose to (E, T*B), col = t*B + b
    xs = embed[target.astype(np.int64)]  # (B, T, E)
    xsT = np.ascontiguousarray(xs.transpose(1, 0, 2).reshape(TB, E).T)  # (E, TB)
    xsT_pad = np.zeros((128, TB), f)
    xsT_pad[:E] = xsT
    xsT_pad[E] = 1.0  # bias row

    bias_gi = np.concatenate([b_ih[: 2 * H] + b_hh[: 2 * H], b_ih[2 * H :]])
    wihT_pad = np.zeros((128, 3 * H), f)
    wihT_pad[:E] = W_ih.T
    wihT_pad[E] = bias_gi

    whhT = np.ascontiguousarray(W_hh.T)  # (H, 3H)
    bhhn = np.ascontiguousarray(b_hh[2 * H :][None, :])  # (1, H)

    stT_pad = np.zeros((KST, B), f)
    stT_pad[:INPUT_DIM] = state.T
    stT_pad[INPUT_DIM] = 1.0
    wpT_pad = np.zeros((KST, H), f)
    wpT_pad[:INPUT_DIM] = Wp.T
    wpT_pad[INPUT_DIM] = bp

    woT_full = np.zeros((H, VPAD), f)
    woT_full[:, :V] = Wo.T
    bo_full = np.zeros((VPAD,), f)
    bo_full[:V] = bo

    cb = np.zeros((1, 256), ml_dtypes.bfloat16)
    cb[0, 128:] = 1.0

    in_maps = []
    for c in range(NCORES):
        vs = slice(c * VLOC, (c + 1) * VLOC)
        in_maps.append(
            {
                "xsT": xsT_pad,
                "wihT": wihT_pad,
                "whhT": whhT,
                "bhhn": bhhn,
                "stT": stT_pad,
                "wpT": wpT_pad,
                "woT": np.ascontiguousarray(woT_full[:, vs]),
                "bo": np.ascontiguousarray(bo_full[vs].reshape(NVCH, 128).T),
                "cb": cb,
            }
        )
    return in_maps


def _assemble(results):
    full = np.concatenate([r["out"] for r in results], axis=0)  # (VPAD, TB)
    # out[b, t, v] = full[v, t*B + b]
    out = full[:V].reshape(V, T, B).transpose(2, 1, 0)
    return np.ascontiguousarray(out)


def _run(in_maps, **kwargs):
    from concourse.bass_utils import run_bass_kernel_spmd

    nc = _get_nc()
    return run_bass_kernel_spmd(nc, in_maps, core_ids=list(range(NCORES)), **kwargs)


def kernel(**inputs):
    in_maps = _prep_in_maps(**inputs)
    res = _run(in_maps)
    return _assemble(res.results)


# revision 48
# speedup vs baseline: 1.0286x; 1.0218x over previous
"""Trainium2 Bass kernel for a GRU "communication head".

Model (per reference):
    h0 = state @ Wp.T + bp                    (B, H)
    xs = embed[target]                        (B, T, E)
    for t: h = GRUCell(xs[:, t], h); logits_t = h @ Wo.T + bo
    out = stack logits                        (B, T, V)

Shapes: B=32, T=64, E=64, H=256, V=32003, INPUT_DIM=512.

Strategy (8 NeuronCores):
  - Vocab(column)-parallel: V padded to 32768 = 8 * 4096. Each core holds a
    4096-wide slice of Wo/bo and produces logits for ALL (t, b) positions of
    its slice. The tiny GRU recurrence is computed redundantly on every core.
  - Transposed on-chip layout: hidden states live as H_allT [128, 2, T*B]
    (f32r) so the output projection is a dense fp32r matmul (full PE rate)
    streamed to HBM as it becomes available.
  - Output is vocab-major ([v, t*B+b] per core) so bo is a per-partition
    bias fused for free into the PSUM->SBUF copy (DVE tensor_scalar_add /
    ACT Identity+bias, alternating to balance the two engines).
  - The recurrence runs as TWO independent 16-wide batch streams,
    software-pipelined at half-step "tick" granularity so neither stream
    head-of-line blocks the other on the in-order engine queues; the
    h-side (W_hh @ h) matmuls consume H_allT directly in f32r.
  - Gate pre-activations accumulate in one PSUM bank per (step, stream):
    slots [r, z, h_n, i_n] x 2 H-chunks x 16. A K=1 bf16 "zero opener"
    matmul clears the bank and orders the accumulation; x-side fp32
    contributions (incl. all biases via an appended ones-row on the
    embedded inputs) and the h-side matmuls then accumulate.
  - Output-projection tiles are emitted interleaved (~1.5 per tick) so the
    in-order PE queue never stalls the recurrence; first/last waves are
    256 columns so the output DMA stream starts early and drains early.
"""

import numpy as np
import ml_dtypes

B = 32
SB = 16  # per-stream batch
T = 64
E = 64
H = 256
V = 32003
INPUT_DIM = 512
NCORES = 8
VPAD = 32768
VLOC = VPAD // NCORES  # 4096
TB = T * B  # 2048
KST = 640  # padded (INPUT_DIM + bias row) -> 5 chunks of 128
NVCH = VLOC // 128  # 32 vocab chunks (output partition dim)
DMA_GROUP = 4  # vocab chunks per output DMA
# (start_col, ncols) waves; ends when the step producing its last column done
WAVES = [(0, 256), (256, 256), (512, 512), (1024, 512), (1536, 256), (1792, 256)]

_CACHE = {}


def _build_nc():
    import concourse.mybir as mybir
    import concourse.tile as tile
    from concourse import bacc

    f32 = mybir.dt.float32
    f32r = mybir.dt.float32r
    bf16 = mybir.dt.bfloat16
    AF = mybir.ActivationFunctionType

    nc = bacc.Bacc(
        "TRN2",
        debug=False,
        enable_asserts=False,
        target_bir_lowering=False,
        num_devices=NCORES,
    )

    d_xsT = nc.dram_tensor("xsT", (128, TB), f32, kind="ExternalInput")
    d_wihT = nc.dram_tensor("wihT", (128, 3 * H), f32, kind="ExternalInput")
    d_whhT = nc.dram_tensor("whhT", (H, 3 * H), f32r, kind="ExternalInput")
    d_bhhn = nc.dram_tensor("bhhn", (1, H), f32, kind="ExternalInput")
    d_stT = nc.dram_tensor("stT", (KST, B), f32, kind="ExternalInput")
    d_wpT = nc.dram_tensor("wpT", (KST, H), f32, kind="ExternalInput")
    d_woT = nc.dram_tensor("woT", (H, VLOC), f32r, kind="ExternalInput")
    # bo regrouped host-side to [128, NVCH]: column m holds bo[m*128:(m+1)*128]
    d_bo = nc.dram_tensor("bo", (128, NVCH), f32, kind="ExternalInput")
    # [1, 256] bf16: cols 0:128 zeros, 128:256 ones (for the PSUM openers)
    d_cb = nc.dram_tensor("cb", (1, 256), bf16, kind="ExternalInput")
    # vocab-major output: row v (local), col t*B+b
    d_out = nc.dram_tensor("out", (VLOC, TB), f32, kind="ExternalOutput")

    with tile.TileContext(nc) as tc:
        with (
            tc.tile_pool(name="weights", bufs=1) as wpool,
            tc.tile_pool(name="state", bufs=1) as spool,
            tc.tile_pool(name="gates_ps", bufs=2, space="PSUM") as gps,
            tc.tile_pool(name="logit_ps", bufs=4, space="PSUM") as lps,
            tc.tile_pool(name="tmp", bufs=24) as tmp,
            tc.tile_pool(name="ostage", bufs=6) as ost,
        ):
            # ---- persistent SBUF loads ----
            # order matters: the opener constants (cb) and the recurrence
            # weights must not queue behind the 4MB woT load, or the first
            # recurrence steps stall ~28us waiting for them
            cb = wpool.tile([1, 256], bf16, tag="cb")
            nc.gpsimd.dma_start(out=cb, in_=d_cb.ap())
            stT = wpool.tile([128, 5, B], f32, tag="stT")
            nc.sync.dma_start(
                out=stT, in_=d_stT.ap().rearrange("(kc p) b -> p kc b", p=128)
            )
            wpT = wpool.tile([128, 5, H], f32, tag="wpT")
            nc.sync.dma_start(
                out=wpT, in_=d_wpT.ap().rearrange("(kc p) m -> p kc m", p=128)
            )
            bhhn = wpool.tile([1, H], f32, tag="bhhn")
            nc.gpsimd.dma_start(out=bhhn, in_=d_bhhn.ap())
            wihT = wpool.tile([128, 3 * H], f32, tag="wihT")
            nc.sync.dma_start(out=wihT, in_=d_wihT.ap())
            whhT = wpool.tile([128, 2, 3 * H], f32r, tag="whhT")
            nc.sync.dma_start(
                out=whhT, in_=d_whhT.ap().rearrange("(kc p) m -> p kc m", p=128)
            )
            xsT = wpool.tile([128, TB], f32, tag="xsT")
            for c4 in range(4):
                cs4 = slice(c4 * (TB // 4), (c4 + 1) * (TB // 4))
                nc.sync.dma_start(out=xsT[:, cs4], in_=d_xsT.ap()[:, cs4])
            bo = wpool.tile([128, NVCH], f32, tag="bo")
            nc.gpsimd.dma_start(out=bo, in_=d_bo.ap())
            woT = wpool.tile([128, 2, VLOC], f32r, tag="woT")
            woT_src = d_woT.ap().rearrange("(kc p) v -> p kc v", p=128)
            for c4 in range(4):
                vs4 = slice(c4 * (VLOC // 4), (c4 + 1) * (VLOC // 4))
                nc.sync.dma_start(out=woT[:, :, vs4], in_=woT_src[:, :, vs4])
            zcol_b = cb[0:1, 0:128]
            ones_b = cb[0:1, 128:256]

            ones = wpool.tile([1, 256], f32, tag="ones")
            nc.vector.memset(ones, 1.0)

            H_allT = spool.tile([128, 2, TB], f32r, tag="H_allT")
            h0 = spool.tile([128, 2, B], f32r, tag="h0")

            # vocab-major view of the output for grouped DMA
            out_g = d_out.ap().rearrange("(g p) t -> p g t", p=128)

            # ---- h0 = state @ Wp.T + bp (bias folded into padded row 512) ----
            ps0 = gps.tile([128, 2, B], f32, tag="g0")
            nc.tensor.matmul(
                ps0, zcol_b, ones_b[0:1, 0:64], start=True, stop=False,
                skip_group_check=True,
            )
            for ko in range(2):
                for ki in range(5):
                    nc.tensor.matmul(
                        ps0[:, ko, :],
                        wpT[:, ki, ko * 128 : (ko + 1) * 128],
                        stT[:, ki, :],
                        start=False,
                        stop=(ko == 1 and ki == 4),
                        skip_group_check=True,
                    )
            nc.any.tensor_copy(h0, ps0)

            # ---- output-projection tile machinery ----
            # matmuls are emitted at tick k; the PSUM->SBUF copy (+bias) and
            # the grouped DMA are deferred to tick k+1 so they enter the
            # DVE/ACT queues with their dependencies already satisfied
            # (otherwise they convoy the next tick's recurrence ops).
            state_d = {"flip": 0, "ob": None}
            pending = []  # (start_col, ncols, m) logit tiles ready to emit
            deferred = []  # (ps, m, tbs, ncols) awaiting copy emission

            def emit_logit_mm():
                if not pending:
                    return
                c0, ncols, m = pending.pop(0)
                tbs = slice(c0, c0 + ncols)
                ps = lps.tile([128, ncols], f32, tag="lg", name="lg")
                nc.tensor.matmul(
                    ps,
                    woT[:, 0, m * 128 : (m + 1) * 128],
                    H_allT[:, 0, tbs],
                    start=True,
                    stop=False,
                )
                nc.tensor.matmul(
                    ps,
                    woT[:, 1, m * 128 : (m + 1) * 128],
                    H_allT[:, 1, tbs],
                    start=False,
                    stop=True,
                )
                deferred.append((ps, m, tbs, ncols))

            def emit_logit_copies():
                while deferred:
                    ps, m, tbs, ncols = deferred.pop(0)
                    g, j = divmod(m, DMA_GROUP)
                    if j == 0:
                        state_d["ob"] = ost.tile(
                            [128, DMA_GROUP, ncols], f32, tag="ob", name="ob"
                        )
                    ob = state_d["ob"]
                    bias_ap = bo[:, m : m + 1]
                    if state_d["flip"] % 3 == 0:
                        nc.vector.tensor_scalar_add(ob[:, j, :], ps, bias_ap)
                    else:
                        nc.scalar.activation(
                            out=ob[:, j, :], in_=ps, func=AF.Identity, bias=bias_ap
                        )
                    state_d["flip"] += 1
                    if j == DMA_GROUP - 1:
                        nc.sync.dma_start(
                            out=out_g[
                                :, g * DMA_GROUP : (g + 1) * DMA_GROUP, tbs
                            ],
                            in_=ob,
                        )

            wave_by_end_step = {
                (c0 + ncols) // B - 1: (c0, ncols) for c0, ncols in WAVES
            }

            # ---- recurrence: 2 streams, software-pipelined over "ticks" ----
            # tick k handles stream k%2, step k//2. Per tick we emit, in
            # order: the POST phase (tanh..h-write) of tick k-2, the PRE
            # phase (sigmoid..b2) of tick k-1, and the gate matmuls of tick
            # k. This staggers the two streams' chains by one tick so the
            # in-order engine queues never head-of-line block, and copies
            # slot into the ACT/DVE bubbles.
            NTICK = 2 * T
            pre_st = {}  # tick -> (P, rz, b2, hprev_f, cs)

            def col_slice(k):
                sg, t = k % 2, k // 2
                return slice(t * B + sg * SB, t * B + (sg + 1) * SB)

            def emit_gates(k):
                sg, t = k % 2, k // 2
                cs = col_slice(k)
                if t == 0:
                    hprev_r = h0[:, :, sg * SB : (sg + 1) * SB]
                else:
                    hprev_r = H_allT[
                        :, :, (t - 1) * B + sg * SB : (t - 1) * B + (sg + 1) * SB
                    ]
                hprev_f = hprev_r.bitcast(f32)
                P = gps.tile([128, 4, 2, SB], f32, tag=f"g{sg}", name=f"P{sg}")
                nc.tensor.matmul(
                    P, zcol_b, ones_b, start=True, stop=False,
                    skip_group_check=True,
                )
                for s, slot in ((0, 0), (1, 1), (2, 3)):
                    for ko in range(2):
                        blk = s * 2 + ko
                        nc.tensor.matmul(
                            P[:, slot, ko, :],
                            wihT[:, blk * 128 : (blk + 1) * 128],
                            xsT[:, cs],
                            start=False,
                            stop=False,
                            skip_group_check=True,
                        )
                for ko in range(2):
                    nc.tensor.matmul(
                        P[:, 2, ko, :],
                        bhhn[0:1, ko * 128 : (ko + 1) * 128],
                        ones[0:1, 0:SB],
                        start=False,
                        stop=False,
                        skip_group_check=True,
                    )
                for s in range(3):
                    slot = s if s < 2 else 2
                    for ko in range(2):
                        blk = s * 2 + ko
                        for ki in range(2):
                            nc.tensor.matmul(
                                P[:, slot, ko, :],
                                whhT[:, ki, blk * 128 : (blk + 1) * 128],
                                hprev_r[:, ki, :],
                                start=False,
                                stop=(s == 2 and ko == 1 and ki == 1),
                                skip_group_check=True,
                            )
                pre_st[k] = (P, hprev_f, cs)

            def emit_pre(k):
                sg = k % 2
                P, hprev_f, cs = pre_st[k]
                rz = tmp.tile([128, 2, 2, SB], f32, tag=f"rz{sg}", name=f"rz{sg}")
                nc.scalar.activation(out=rz, in_=P[:, 0:2, :, :], func=AF.Sigmoid)
                a = tmp.tile([128, 2, SB], f32, tag=f"a{sg}", name=f"a{sg}")
                nc.vector.tensor_mul(a, rz[:, 0, :, :], P[:, 2, :, :])
                b2 = tmp.tile([128, 2, SB], f32, tag=f"b2{sg}", name=f"b2{sg}")
                nc.vector.tensor_add(b2, a, P[:, 3, :, :])
                q = tmp.tile([128, 2, SB], f32, tag=f"q{sg}", name=f"q{sg}")
                nc.gpsimd.tensor_mul(q, rz[:, 1, :, :], hprev_f)
                om = tmp.tile([128, 2, SB], f32, tag=f"om{sg}", name=f"om{sg}")
                nc.gpsimd.tensor_scalar(om, rz[:, 1, :, :], -1.0, 1.0,
                                        mybir.AluOpType.mult, mybir.AluOpType.add)
                pre_st[k] = (P, b2, q, om, cs)

            def emit_post(k):
                sg = k % 2
                P, b2, q, om, cs = pre_st.pop(k)
                nsb = tmp.tile([128, 2, SB], f32, tag=f"nsb{sg}", name=f"nsb{sg}")
                nc.scalar.activation(out=nsb, in_=b2, func=AF.Tanh)
                v = tmp.tile([128, 2, SB], f32, tag=f"v{sg}", name=f"v{sg}")
                nc.vector.tensor_mul(v, nsb, om)
                # single f32r master copy: consumed by the next step's
                # h-side matmuls AND the output projection
                nc.vector.tensor_add(H_allT[:, :, cs], v, q)

            for k in range(NTICK + 2):
                if k >= 2:
                    emit_post(k - 2)
                    sg, t = (k - 2) % 2, (k - 2) // 2
                    if sg == 1 and t in wave_by_end_step:
                        c0, ncols = wave_by_end_step[t]
                        pending.extend((c0, ncols, m) for m in range(NVCH))
                emit_logit_copies()
                if 1 <= k <= NTICK:
                    emit_pre(k - 1)
                if k < NTICK:
                    emit_gates(k)
                # ~1.5 logit tiles per tick keeps PE dense without stalling
                # the in-order recurrence matmuls behind a whole wave
                emit_logit_mm()
                if k % 2 == 0:
                    emit_logit_mm()

            while pending:
                emit_logit_mm()
                emit_logit_copies()
            emit_logit_copies()

    nc.compile()
    return nc


def _get_nc():
    if "nc" not in _CACHE:
        _CACHE["nc"] = _build_nc()
    return _CACHE["nc"]


def _prep_in_maps(state, target, embed, Wp, bp, W_ih, W_hh, b_ih, b_hh, Wo, bo):
    f = np.float32
    state = np.asarray(state, dtype=f)
    target = np.asarray(target)
    embed = np.asarray(embed, dtype=f)
    Wp = np.asarray(Wp, dtype=f)
    bp = np.asarray(bp, dtype=f)
    W_ih = np.asarray(W_ih, dtype=f)
    W_hh = np.asarray(W_hh, dtype=f)
    b_ih = np.asarray(b_ih, dtype=f)
    b_hh = np.asarray(b_hh, dtype=f)
    Wo = np.asarray(Wo, dtype=f)
    bo = np.asarray(bo, dtype=f)

    # host-side gather + transpose to (E, T*B), col = t*B + b
    xs = embed[target.astype(np.int64)]  # (B, T, E)
    xsT = np.ascontiguousarray(xs.transpose(1, 0, 2).reshape(TB, E).T)  # (E, TB)
    xsT_pad = np.zeros((128, TB), f)
    xsT_pad[:E] = xsT
    xsT_pad[E] = 1.0  # bias row

    bias_gi = np.concatenate([b_ih[: 2 * H] + b_hh[: 2 * H], b_ih[2 * H :]])
    wihT_pad = np.zeros((128, 3 * H), f)
    wihT_pad[:E] = W_ih.T
    wihT_pad[E] = bias_gi

    whhT = np.ascontiguousarray(W_hh.T)  # (H, 3H)
    bhhn = np.ascontiguousarray(b_hh[2 * H :][None, :])  # (1, H)

    stT_pad = np.zeros((KST, B), f)
    stT_pad[:INPUT_DIM] = state.T
    stT_pad[INPUT_DIM] = 1.0
    wpT_pad = np.zeros((KST, H), f)
    wpT_pad[:INPUT_DIM] = Wp.T
    wpT_pad[INPUT_DIM] = bp

    woT_full = np.zeros((H, VPAD), f)
    woT_full[:, :V] = Wo.T
    bo_full = np.zeros((VPAD,), f)
    bo_full[:V] = bo

    cb = np.zeros((1, 256), ml_dtypes.bfloat16)
    cb[0, 128:] = 1.0

    in_maps = []
    for c in range(NCORES):
        vs = slice(c * VLOC, (c + 1) * VLOC)
        in_maps.append(
            {
                "xsT": xsT_pad,
                "wihT": wihT_pad,
                "whhT": whhT,
                "bhhn": bhhn,
                "stT": stT_pad,
                "wpT": wpT_pad,
                "woT": np.ascontiguousarray(woT_full[:, vs]),
                "bo": np.ascontiguousarray(bo_full[vs].reshape(NVCH, 128).T),
                "cb": cb,
            }
        )
    return in_maps


def _assemble(results):
    full = np.concatenate([r["out"] for r in results], axis=0)  # (VPAD, TB)
    # out[b, t, v] = full[v, t*B + b]
    out = full[:V].reshape(V, T, B).transpose(2, 1, 0)
    return np.ascontiguousarray(out)


def _run(in_maps, **kwargs):
    from concourse.bass_utils import run_bass_kernel_spmd

    nc = _get_nc()
    return run_bass_kernel_spmd(nc, in_maps, core_ids=list(range(NCORES)), **kwargs)


def kernel(**inputs):
    in_maps = _prep_in_maps(**inputs)
    res = _run(in_maps)
    return _assemble(res.results)
